# revision 1
# baseline (speedup 1.0000x reference)
"""Conv2dSubsampling + time-synchronous attention register kernel.

Strategy: data-parallel over batch across the 8 NeuronCores (batch 32 -> 8
shards of 4), per the sharding hint. All ops are batch-independent; weights
are broadcast. The forward pass is executed on the trn2 devices via jax pmap
(PJRT). Falls back to single-device execution if fewer than 8 devices are
visible so the function always returns correct full-shape outputs.
"""

import functools

import numpy as np
import jax
import jax.numpy as jnp

B, T, IDIM = 32, 2048, 80
ODIM, S, H = 256, 256, 8
F2 = ((IDIM - 1) // 2 - 1) // 2  # 19
NCORES = 8


def _conv(x, w, b, stride, groups=1):
    y = jax.lax.conv_general_dilated(
        x, w, (stride, stride), "VALID",
        dimension_numbers=("NCHW", "OIHW", "NCHW"), feature_group_count=groups)
    return y + b[None, :, None, None]


def _posenc(tlen, d):
    pos = jnp.arange(tlen, dtype=jnp.float32)[:, None]
    div = jnp.exp(jnp.arange(0, d, 2, dtype=jnp.float32) * (-np.log(10000.0) / d))
    pe = jnp.zeros((tlen, d), jnp.float32)
    pe = pe.at[:, 0::2].set(jnp.sin(pos * div))
    pe = pe.at[:, 1::2].set(jnp.cos(pos * div))
    return pe


def _forward(x, x_mask, spk_emb, spk_mask, c1w, c1b, c2w, c2b, c3w, c3b,
             ow, ob, wq, bq, wk, bk, wv, bv, wo, bo):
    x = jax.nn.relu(_conv(x, c1w, c1b, 2))
    x = _conv(x, c2w, c2b, 2, groups=8)
    x = jax.nn.relu(_conv(x, c3w, c3b, 1))
    x = x @ ow.T + ob
    m = x_mask[:, :, :-2:2][:, :, :-2:2]
    att_mask = (jnp.swapaxes(m, 1, 2) & spk_mask)[:, None]

    b, L, tp, d = x.shape
    dk = d // H
    q = (x @ wq.T + bq).reshape(b, L, tp, H, dk)
    lev = jnp.arange(H)
    q = jnp.transpose(q[:, lev, :, lev, :], (1, 0, 2, 3))
    k = (spk_emb @ wk.T + bk).reshape(b, -1, H, dk).transpose(0, 2, 1, 3)
    v = (spk_emb @ wv.T + bv).reshape(b, -1, H, dk).transpose(0, 2, 1, 3)
    scores = jnp.einsum("bhtd,bhsd->bhts", q, k) / jnp.sqrt(jnp.float32(dk))
    scores = jnp.where(att_mask, scores, jnp.float32(-1e9))
    attn = jnp.where(att_mask, jax.nn.softmax(scores, axis=-1), 0.0)
    ctx = jnp.einsum("bhts,bhsd->bhtd", attn, v).transpose(0, 2, 1, 3).reshape(b, tp, d)
    y = ctx @ wo.T + bo
    y = y * jnp.sqrt(jnp.float32(d)) + _posenc(tp, d)[None]
    return y, m


_WNAMES = ("conv1_w", "conv1_b", "conv2_w", "conv2_b", "conv3_w", "conv3_b",
           "out_w", "out_b", "wq", "bq", "wk", "bk", "wv", "bv", "wo", "bo")


@functools.lru_cache(maxsize=1)
def _get_pfwd():
    devs = jax.devices()[:NCORES]
    in_axes = (0, 0, 0, 0) + (None,) * len(_WNAMES)
    return jax.pmap(_forward, in_axes=in_axes, devices=devs), len(devs)


def kernel(**inputs):
    arr = {k: np.asarray(v) for k, v in inputs.items()}
    x = arr["x"].astype(np.float32)
    xm = arr["x_mask"].astype(bool)
    se = arr["spk_emb"].astype(np.float32)
    sm = arr["spk_mask"].astype(bool)
    ws = [arr[n].astype(np.float32) for n in _WNAMES]

    try:
        pfwd, nd = _get_pfwd()
        if nd < NCORES:
            raise RuntimeError("need 8 cores")
        bl = B // NCORES
        xs = x.reshape(NCORES, bl, *x.shape[1:])
        xms = xm.reshape(NCORES, bl, *xm.shape[1:])
        ses = se.reshape(NCORES, bl, *se.shape[1:])
        sms = sm.reshape(NCORES, bl, *sm.shape[1:])
        y, m = pfwd(xs, xms, ses, sms, *ws)
        y = np.asarray(y).reshape(B, *y.shape[2:])
        m = np.asarray(m).reshape(B, *m.shape[2:])
    except Exception:
        y, m = _forward(jnp.asarray(x), jnp.asarray(xm), jnp.asarray(se),
                        jnp.asarray(sm), *[jnp.asarray(w) for w in ws])
        y, m = np.asarray(y), np.asarray(m)
    return y.astype(np.float32), m.astype(bool)
